# revision 19
# baseline (speedup 1.0000x reference)
"""Bass/Trainium2 kernel for nn_BiHgru2_1d (bidirectional HGRU block), 8-core SPMD.

Math (reference):
    feat = x @ W_in.T + b_in                    # (N,B,3D)
    inp, og, fg = split(feat); inp=silu(inp); og=sigmoid(og); lam=sigmoid(fg)
    u[h,d,e] = (1-lam[h,d]) * inp[h,e];  lam_f[h,d,e] = lam[h,d]
    s = fwd_scan(lam_f, u) + rev_scan(lam_f, u)         # h_t = lam_t h_{t-1} + u_t
    o[h,e] = sum_d s[h,d,e]*og[h,d]; o = LN(o)*gamma+beta; out = o @ W_out.T + b_out

Sharding: 8-way tensor parallel over heads (128 heads/core). Each core:
  GEMM1 (x full, W_in 768-row slice; inp in fp16, og+fg in fp8 DoubleRow) ->
  activations -> per-(b,d,e) tensor_tensor_scan fwd+rev (rev via negative
  stride) -> o_acc -> per-core partial LN stats -> per-batch AllToAll
  carrying o_acc + stat partials (reshard channel->token) -> GEMM2 with
  gamma folded into W2 and the LN mean/std terms injected as a K=2
  mini-matmul row pair -> per-token scale on the ACT engine -> each core
  writes tokens (all b, n in [256i, 256(i+1))) of the output in f16.

Sign trick: we compute u' = (lam-1)*inp = -u (no rsub on HW), so s' = -s
and o' = -o.  With SUM' = sum_D o' = -D*mu and SSQ' = sum_D o'^2:
    mu = -SUM'/D, var = SSQ'/D - mu^2, sd = sqrt(var+eps), a_t = -1/sd
    out[t,:] = a_t * (G'[t,:] + mu_t*c1 - sd_t*c2)
with G' = o'@W2g (W2g = gamma-scaled W_out.T), c1 = gamma@W_out.T,
c2 = beta@W_out.T + b_out.  The (mu, -sd) pair forms a K=2 stationary row
block multiplied against (c1; c2) inside the GEMM2 accumulation, so the
epilogue is a single per-token ACT scale by a_t.

Queue schedule (in-order queues force this emission order):
  iter b: G1 blocks(b) | stats-chain(b-1) | stat-reduce(b-2) | scans(b)
  pass 2: G2(0) G2(1) | chain(3) | reduce(2) reduce(3) | G2(2) G2(3)
so no engine ever blocks another batch's producers.
"""

import sys

for _p in ("/opt/trn_rl_repo",):
    if _p not in sys.path:
        sys.path.insert(0, _p)

import numpy as np

# ---- problem constants (hardcoded per contract) ----
N_FULL, B, D = 2048, 4, 2048
E = 2
H = D // E                      # 1024 heads
NCORES = 8
P = 128                         # partitions
HC = H // NCORES                # 128 heads per core
KC = D // P                     # 16 k-chunks
M_TILES = 6                     # [inp e0, inp e1, og e0, og e1, fg d0, fg d1]

OG_FULL8 = True                 # og path fully fp8 (else half fp8 / half f16)

_BUILD_CACHE = {}


def build_program(T=N_FULL, num_devices=NCORES, og_full8=OG_FULL8,
                  use_silu=True, debug_dump=False):
    """Build the SPMD Bass program (same program on every core)."""
    import concourse.bass as bass
    import concourse.mybir as mybir
    import concourse.tile as tile
    from concourse import bacc

    f16 = mybir.dt.float16
    f32 = mybir.dt.float32
    fp8 = mybir.dt.float8e4
    MUL = mybir.AluOpType.mult
    ADD = mybir.AluOpType.add
    SUB = mybir.AluOpType.subtract
    AF = mybir.ActivationFunctionType
    DR = mybir.MatmulPerfMode.DoubleRow

    NSEG = T // NCORES           # per-core seq positions per batch (256)
    TOK_C = B * NSEG             # tokens per core after reshard (1024)
    NBLK = min(512, T)           # GEMM1 token-block size (per batch)
    NB1 = T // NBLK              # token blocks per batch (4)
    TCH = min(P, NSEG)           # GEMM2 token-chunk (output partition dim)
    NTCH = NSEG // TCH           # token chunks per batch (2)
    OCB = 256                    # GEMM2 out-col block
    NOC = D // OCB               # 8
    KD = KC // 2                 # fp8 DoubleRow k-pairs (8)
    NM16 = 2 if og_full8 else 4  # f16 m-tiles (inp; +og halves if not full8)
    M8 = 4 if og_full8 else 2    # fp8 m-tiles (og+fg or fg only)
    CH = 2 * NSEG                # stat partial chunk (512 tokens)
    NCH = T // CH                # 4
    PAY = P * E * NSEG           # oac payload elems per A2A destination
    assert T % (NCORES * TCH) == 0 and T % NBLK == 0

    nc = bacc.Bacc("TRN2", target_bir_lowering=False, debug=False,
                   num_devices=num_devices)

    # ---- per-core DRAM parameters ----
    xT_d = nc.dram_tensor("xT", [D, B * T], f16, kind="ExternalInput")
    xT8_d = nc.dram_tensor("xT8", [D, B * T], fp8, kind="ExternalInput")
    w1T_d = nc.dram_tensor("w1T", [D, NM16 * P], f16, kind="ExternalInput")
    w18_d = nc.dram_tensor("w18", [D, M8 * P], fp8, kind="ExternalInput")
    if not og_full8:
        w18og_d = nc.dram_tensor("w18og", [D // 2, 2 * P], fp8,
                                 kind="ExternalInput")
    b1_d = nc.dram_tensor("b1", [P, M_TILES], f32, kind="ExternalInput")
    w2g_d = nc.dram_tensor("w2g", [D, D], f16, kind="ExternalInput")
    c12_d = nc.dram_tensor("c12", [2, D], f16, kind="ExternalInput")
    out_d = nc.dram_tensor("out", [TOK_C, D], f16, kind="ExternalOutput")
    if debug_dump:
        dbg_act = nc.dram_tensor("dbg_act", [3, P, E, T], f16,
                                 kind="ExternalOutput")
        dbg_oac = nc.dram_tensor("dbg_oac", [B, P, E, T], f16,
                                 kind="ExternalOutput")
        dbg_ot = nc.dram_tensor("dbg_ot", [B, P, KC, NSEG], f16,
                                 kind="ExternalOutput")

    xT_r = xT_d.ap().rearrange("(kc p) t -> p kc t", p=P)
    xT8_r = xT8_d.ap().rearrange("(kd ko p) t -> p kd ko t", p=P, ko=2)
    w1T_r = w1T_d.ap().rearrange("(kc p) m -> p kc m", p=P)
    w18_r = w18_d.ap().rearrange("(kd ko p) m -> p kd ko m", p=P, ko=2)
    if not og_full8:
        w18og_r = w18og_d.ap().rearrange("(kd ko p) m -> p kd ko m", p=P, ko=2)
    w2g_r = w2g_d.ap().rearrange("(kc p) o -> p kc o", p=P)

    with tile.TileContext(nc) as tc:
        with (
            tc.tile_pool(name="cst", bufs=1) as cst_pool,
            tc.tile_pool(name="w1p", bufs=1) as w1_pool,
            tc.tile_pool(name="xs", bufs=2) as x_pool,
            tc.tile_pool(name="res", bufs=2) as res_pool,
            tc.tile_pool(name="oacp", bufs=2) as oac_pool,
            tc.tile_pool(name="up", bufs=1) as u_pool,
            tc.tile_pool(name="scan", bufs=2) as scan_pool,
            tc.tile_pool(name="otp", bufs=3) as ot_pool,
            tc.tile_pool(name="w2p", bufs=2) as w2_pool,
            tc.tile_pool(name="sqp", bufs=2) as sq_pool,
            tc.tile_pool(name="stp", bufs=2) as stp_pool,
            tc.tile_pool(name="epi", bufs=2) as epi_pool,
            tc.tile_pool(name="ps", bufs=2, space="PSUM") as psum_pool,
            tc.tile_pool(name="dram", bufs=2, space="DRAM") as dram_pool,
        ):
            # ---- constants (small, scalar queue) ----
            b1_sb = cst_pool.tile([P, M_TILES], f32, tag="b1")
            nc.scalar.dma_start(b1_sb[:], b1_d.ap())
            ones_sb = cst_pool.tile([P, 1], f16, tag="ones")
            nc.vector.memset(ones_sb[:], 1.0)
            eps_sb = cst_pool.tile([P, 1], f32, tag="eps")
            nc.vector.memset(eps_sb[:], 1e-5)
            cc12_sb = cst_pool.tile([2, D], f16, tag="cc12")
            nc.scalar.dma_start(cc12_sb[:], c12_d.ap())

            # ---- weights + batch0/block0 x, interleaved kc-wise so the
            # m-inner first block can chase the DMA stream ----
            w1_sb = w1_pool.tile([P, KC, NM16 * P], f16, tag="w1")
            w18_sb = w1_pool.tile([P, KD, 2, M8 * P], fp8, tag="w18")
            xt0 = x_pool.tile([P, KC, NBLK], f16, tag="xt", name="xt_0_0")
            xt80 = x_pool.tile([P, KD, 2, NBLK], fp8, tag="xt8", name="xt8_0_0")
            for kc in range(KC):
                nc.sync.dma_start(w1_sb[:, kc:kc + 1, :], w1T_r[:, kc:kc + 1, :])
                nc.sync.dma_start(xt0[:, kc:kc + 1, :], xT_r[:, kc:kc + 1, 0:NBLK])
                if kc % 2 == 1:
                    kd = kc // 2
                    nc.sync.dma_start(w18_sb[:, kd:kd + 1, :, :],
                                      w18_r[:, kd:kd + 1, :, :])
                    nc.sync.dma_start(xt80[:, kd:kd + 1, :, :],
                                      xT8_r[:, kd:kd + 1, :, 0:NBLK])
            if not og_full8:
                w18og_sb = w1_pool.tile([P, KD // 2, 2, 2 * P], fp8, tag="w18og")
                nc.sync.dma_start(w18og_sb[:], w18og_r)

            # per-batch live tile refs
            res_tiles = [None] * B      # (lam, inp, og, oac)
            ot_tiles = [None] * B
            stT8_tiles = [None] * B
            aT_tiles = [None] * B
            mrow_tiles = [None] * B     # [2, NTCH, TCH] f16 (mu; -sd) rows
            cc_tiles = [None] * B

            # GEMM1 m-tile epilogue: m 0,1 -> inp e0,e1 (silu);
            # 2,3 -> og e0,e1; 4,5 -> fg d0,d1 (sigmoid, 1/16 descale)
            def g1_epilogue(b, m, ps, nb):
                lam_b, inp_b, og_b, _ = res_tiles[b]
                dest, func, scale = [
                    (inp_b, AF.Silu, 1.0), (inp_b, AF.Silu, 1.0),
                    (og_b, AF.Sigmoid, 1.0 / 16.0),
                    (og_b, AF.Sigmoid, 1.0 / 16.0),
                    (lam_b, AF.Sigmoid, 1.0 / 16.0),
                    (lam_b, AF.Sigmoid, 1.0 / 16.0)][m]
                dsl = dest[:, m % 2, nb * NBLK:(nb + 1) * NBLK]
                if func == AF.Silu and not use_silu:
                    sg = sq_pool.tile([P, NBLK], f32, tag="sg")
                    nc.scalar.activation(sg[:], ps[:], AF.Sigmoid,
                                         bias=b1_sb[:, m:m + 1], scale=scale)
                    nc.scalar.activation(ps[:], ps[:], AF.Identity,
                                         bias=b1_sb[:, m:m + 1], scale=scale)
                    nc.vector.tensor_tensor(dsl, ps[:], sg[:], MUL)
                else:
                    nc.scalar.activation(dsl, ps[:], func,
                                         bias=b1_sb[:, m:m + 1], scale=scale)

            def emit_g1_block(b, nb):
                tok0 = b * T + nb * NBLK
                if b == 0 and nb == 0:
                    xt, xt8 = xt0, xt80
                else:
                    xt = x_pool.tile([P, KC, NBLK], f16, tag="xt",
                                     name=f"xt_{b}_{nb}")
                    xt8 = x_pool.tile([P, KD, 2, NBLK], fp8, tag="xt8",
                                      name=f"xt8_{b}_{nb}")
                    for q in range(4):
                        nc.sync.dma_start(
                            xt[:, 4 * q:4 * (q + 1), :],
                            xT_r[:, 4 * q:4 * (q + 1), tok0:tok0 + NBLK])
                    for q in range(2):
                        nc.sync.dma_start(
                            xt8[:, 4 * q:4 * (q + 1), :, :],
                            xT8_r[:, 4 * q:4 * (q + 1), :, tok0:tok0 + NBLK])
                if b == 0 and nb == 0:
                    # m-inner, kc-outer in two 3-tile groups: chases the
                    # interleaved DMA stream with 3 PSUM groups at once.
                    assert og_full8, "m-inner fast start needs og_full8"
                    for grp in ((0, 1, 2), (3, 4, 5)):
                        pss = {m: psum_pool.tile([P, NBLK], f32, tag="ps",
                                                 bufs=4, name=f"ps0_{m}")
                               for m in grp}
                        for kc in range(KC):
                            for m in grp:
                                if m < NM16:
                                    nc.tensor.matmul(
                                        pss[m][:],
                                        w1_sb[:, kc, m * P:(m + 1) * P],
                                        xt[:, kc, :],
                                        start=(kc == 0), stop=(kc == KC - 1))
                                elif kc % 2 == 1:
                                    kd = kc // 2
                                    m8 = m - NM16
                                    nc.tensor.matmul(
                                        pss[m][:],
                                        w18_sb[:, kd, :, m8 * P:(m8 + 1) * P],
                                        xt8[:, kd, :, :],
                                        start=(kd == 0), stop=(kd == KD - 1),
                                        perf_mode=DR)
                        for m in grp:
                            g1_epilogue(b, m, pss[m], nb)
                    return
                for m in range(M_TILES):
                    ps = psum_pool.tile([P, NBLK], f32, tag="ps", bufs=4,
                                        name=f"ps_{b}_{nb}_{m}")
                    if (og_full8 and m >= 2) or (not og_full8 and m >= 4):
                        m8 = m - NM16
                        for kd in range(KD):
                            nc.tensor.matmul(
                                ps[:], w18_sb[:, kd, :, m8 * P:(m8 + 1) * P],
                                xt8[:, kd, :, :],
                                start=(kd == 0), stop=(kd == KD - 1),
                                perf_mode=DR)
                    elif not og_full8 and m >= 2:
                        for kd in range(KD // 2):
                            nc.tensor.matmul(
                                ps[:],
                                w18og_sb[:, kd, :, (m - 2) * P:(m - 1) * P],
                                xt8[:, kd, :, :],
                                start=(kd == 0), stop=False, perf_mode=DR)
                        for kc in range(KC // 2, KC):
                            nc.tensor.matmul(
                                ps[:], w1_sb[:, kc, m * P:(m + 1) * P],
                                xt[:, kc, :], start=False, stop=(kc == KC - 1))
                    else:
                        for kc in range(KC):
                            nc.tensor.matmul(
                                ps[:], w1_sb[:, kc, m * P:(m + 1) * P],
                                xt[:, kc, :],
                                start=(kc == 0), stop=(kc == KC - 1))
                    g1_epilogue(b, m, ps, nb)

            def emit_scans(b):
                lam_b, inp_b, og_b, oac_b = res_tiles[b]
                u_t = [[u_pool.tile([P, T], f16, tag=f"u{d}{e}",
                                    name=f"u_{b}_{d}{e}")
                        for e in range(E)] for d in range(E)]
                for d in range(E):
                    for e in range(E):
                        nc.vector.scalar_tensor_tensor(
                            u_t[d][e][:], lam_b[:, d, :], 1.0, inp_b[:, e, :],
                            op0=SUB, op1=MUL)
                for rev in (False, True):
                    for d in range(E):
                        lam_bd = lam_b[:, d, :]
                        og_bd = og_b[:, d, :]
                        for e in range(E):
                            s = scan_pool.tile([P, T], f16, tag="s",
                                               name=f"s_{b}_{rev}_{d}{e}")
                            if rev:
                                nc.vector.tensor_tensor_scan(
                                    s[:, ::-1], lam_bd[:, ::-1],
                                    u_t[d][e][:, ::-1], 0.0, op0=MUL, op1=ADD)
                            else:
                                nc.vector.tensor_tensor_scan(
                                    s[:], lam_bd, u_t[d][e][:], 0.0,
                                    op0=MUL, op1=ADD)
                            o_be = oac_b[:, e, :]
                            if not rev and d == 0:
                                nc.vector.tensor_tensor(o_be, og_bd, s[:], MUL)
                            else:
                                nc.vector.tensor_tensor(s[:], og_bd, s[:], MUL)
                                nc.vector.tensor_tensor(o_be, o_be, s[:], ADD)
                if debug_dump:
                    nc.gpsimd.dma_start(dbg_oac.ap()[b], oac_b[:])
                    if b == 0:
                        nc.gpsimd.dma_start(dbg_act.ap()[0], lam_b[:])
                        nc.gpsimd.dma_start(dbg_act.ap()[1], inp_b[:])
                        nc.gpsimd.dma_start(dbg_act.ap()[2], og_b[:])

            def emit_chain(b):
                """Partial stats + A2A + ot/stat gathers for batch b.
                Emitted after G1(b+1) so the tensor/ACT queues reach the
                stat ops only once oac(b) is (nearly) ready."""
                _, inp_b, _, oac_b = res_tiles[b]
                # sqac reuses inp_b (dead once u' tiles exist)
                nc.scalar.square(inp_b[:], oac_b[:])
                cc_in = dram_pool.tile([NCORES, PAY + 2 * NSEG], f16,
                                       tag="cc_in", name=f"cc_in_{b}")
                cc_out = dram_pool.tile([NCORES, PAY + 2 * NSEG], f16,
                                        tag="cc_out", name=f"cc_out_{b}")
                cc_tiles[b] = cc_out
                cc_oac_in = cc_in[:, 0:PAY].rearrange(
                    "j (p e t) -> j p e t", p=P, e=E)
                cc_st_in = cc_in[:, PAY:].rearrange("j (s t) -> j s t", s=2)
                # stat partial matmuls (sequential accumulation groups) +
                # psum->sbuf f16 copies
                stcs = []
                for c in range(NCH):
                    ts = slice(c * CH, (c + 1) * CH)
                    for si, srct in ((0, oac_b), (1, inp_b)):
                        psc = psum_pool.tile([1, CH], f32, tag="pst", bufs=2,
                                             name=f"pst_{b}_{c}_{si}")
                        nc.tensor.matmul(psc[:], ones_sb[:], srct[:, 0, ts],
                                         start=True, stop=False)
                        nc.tensor.matmul(psc[:], ones_sb[:], srct[:, 1, ts],
                                         start=False, stop=True)
                        stc = stp_pool.tile([1, CH], f16, tag="stc", bufs=4,
                                            name=f"stc_{b}_{c}_{si}")
                        nc.scalar.copy(stc[:], psc[:])
                        stcs.append((c, si, stc))
                # gpsimd: oac staging, stat staging, collective, gathers
                for j in range(NCORES):
                    nc.gpsimd.dma_start(
                        cc_oac_in[j], oac_b[:, :, j * NSEG:(j + 1) * NSEG])
                for c, si, stc in stcs:
                    for h in range(2):
                        nc.gpsimd.dma_start(
                            cc_st_in[2 * c + h, si, :],
                            stc[:1, h * NSEG:(h + 1) * NSEG])
                nc.gpsimd.collective_compute(
                    "AllToAll", mybir.AluOpType.bypass,
                    replica_groups=[list(range(NCORES))],
                    ins=[cc_in.opt()], outs=[cc_out.opt()])
                cc_oac_out = cc_out[:, 0:PAY].rearrange(
                    "j (pe t) -> j pe t", pe=P * E)
                ot = ot_pool.tile([P, KC, NSEG], f16, tag="ot",
                                  name=f"ot_{b}")
                for kc in range(KC):
                    nc.gpsimd.dma_start(
                        ot[:, kc, :],
                        cc_oac_out[kc // 2, (kc % 2) * P:(kc % 2 + 1) * P, :])
                if debug_dump:
                    nc.gpsimd.dma_start(dbg_ot.ap()[b], ot[:])
                ot_tiles[b] = ot
                # stat partial gather: [tok-part, s, j, c]
                stT8 = stp_pool.tile([TCH, 2, NCORES, NTCH], f16, tag="stT8",
                                     name=f"stT8_{b}")
                for s in range(2):
                    for c in range(NTCH):
                        st8 = cc_out[:, PAY + s * NSEG + c * TCH:
                                     PAY + s * NSEG + (c + 1) * TCH].rearrange(
                            "j p -> p j", p=TCH)
                        nc.gpsimd.dma_start(stT8[:, s, :, c], st8)
                stT8_tiles[b] = stT8

            def emit_reduce(b):
                """Reduce stat partials over cores; produce aT (token-part
                scale) and the (mu, -sd) stationary rows via a DRAM bounce."""
                stT8 = stT8_tiles[b]
                nc.vector.tensor_tensor(stT8[:, :, 0:4, :], stT8[:, :, 0:4, :],
                                        stT8[:, :, 4:8, :], ADD)
                nc.vector.tensor_tensor(stT8[:, :, 0:2, :], stT8[:, :, 0:2, :],
                                        stT8[:, :, 2:4, :], ADD)
                stT = stp_pool.tile([TCH, 2, NTCH], f32, tag="stT",
                                    name=f"stT_{b}")
                nc.vector.tensor_tensor(stT[:], stT8[:, :, 0, :],
                                        stT8[:, :, 1, :], ADD)
                # mb[:, c, 0] = mu ; mb[:, c, 1] = -sd ; aT = -1/sd
                mb = stp_pool.tile([TCH, NTCH, 2], f32, tag="mb",
                                   name=f"mb_{b}")
                sd = stp_pool.tile([TCH, 2, NTCH], f32, tag="sd",
                                   name=f"sd_{b}")
                VAR, REC = 0, 1
                nc.vector.tensor_scalar(mb[:, :, 0], stT[:, 0, :], -1.0 / D,
                                        None, op0=MUL)
                nc.vector.tensor_tensor(sd[:, VAR], mb[:, :, 0], mb[:, :, 0],
                                        MUL)
                nc.vector.tensor_scalar(sd[:, REC], stT[:, 1, :], 1.0 / D,
                                        None, op0=MUL)
                nc.vector.tensor_tensor(sd[:, VAR], sd[:, REC], sd[:, VAR],
                                        SUB)
                nc.scalar.activation(sd[:, VAR], sd[:, VAR], AF.Sqrt,
                                     bias=eps_sb[:])
                nc.vector.tensor_scalar(mb[:, :, 1], sd[:, VAR], -1.0, None,
                                        op0=MUL)
                aT = stp_pool.tile([TCH, NTCH], f32, tag="aT",
                                   name=f"aT_{b}")
                nc.vector.reciprocal(sd[:, REC], sd[:, VAR])
                nc.vector.tensor_scalar(aT[:], sd[:, REC], -1.0, None,
                                        op0=MUL)
                aT_tiles[b] = aT
                # bounce (mu, -sd) to row layout [2, NTCH, TCH] f16
                mr_dram = dram_pool.tile([2, NSEG], f32, tag="mrd",
                                         name=f"mrd_{b}")
                mr_view = mr_dram.rearrange("q (c p) -> q p c", p=TCH)
                for q in range(2):
                    nc.gpsimd.dma_start(mr_view[q], mb[:, :, q])
                mrow = stp_pool.tile([2, NTCH, TCH], f16, tag="mrow",
                                     name=f"mrow_{b}")
                nc.gpsimd.dma_start(mrow[:], mr_dram.rearrange(
                    "q (c p) -> q c p", p=TCH))
                mrow_tiles[b] = mrow

            def emit_g2(b):
                ot = ot_tiles[b]
                for oc in range(NOC):
                    ocs = slice(oc * OCB, (oc + 1) * OCB)
                    w2sb = w2_pool.tile([P, KC, OCB], f16, tag="w2",
                                        name=f"w2_{b}_{oc}")
                    for q in range(4):
                        nc.sync.dma_start(w2sb[:, 4 * q:4 * (q + 1), :],
                                          w2g_r[:, 4 * q:4 * (q + 1), ocs])
                    for tch in range(NTCH):
                        ps2 = psum_pool.tile([TCH, OCB], f32, tag="ps2",
                                             bufs=2,
                                             name=f"ps2_{b}_{oc}_{tch}")
                        for kc in range(KC):
                            nc.tensor.matmul(
                                ps2[:], ot[:, kc, tch * TCH:(tch + 1) * TCH],
                                w2sb[:, kc, :], start=(kc == 0), stop=False)
                        nc.tensor.matmul(
                            ps2[:], mrow_tiles[b][:, tch, :],
                            cc12_sb[:, ocs], start=False, stop=True)
                        ob = epi_pool.tile([TCH, OCB], f16, tag="ob",
                                           name=f"ob_{b}_{oc}_{tch}")
                        nc.scalar.activation(
                            ob[:], ps2[:], AF.Copy,
                            scale=aT_tiles[b][:, tch:tch + 1])
                        nc.scalar.dma_start(
                            out_d.ap()[b * NSEG + tch * TCH:
                                       b * NSEG + (tch + 1) * TCH, ocs],
                            ob[:])

            # ================= emission schedule =================
            for b in range(B):
                lam_b = res_pool.tile([P, E, T], f16, tag="lam",
                                      name=f"lam_{b}")
                inp_b = res_pool.tile([P, E, T], f16, tag="inp",
                                      name=f"inp_{b}")
                og_b = res_pool.tile([P, E, T], f16, tag="og",
                                     name=f"og_{b}")
                oac_b = oac_pool.tile([P, E, T], f16, tag="oac",
                                      name=f"oac_{b}")
                res_tiles[b] = (lam_b, inp_b, og_b, oac_b)
                for nb in range(NB1):
                    emit_g1_block(b, nb)
                if b >= 1:
                    emit_chain(b - 1)
                if b >= 2:
                    emit_reduce(b - 2)
                emit_scans(b)
            emit_g2(0)
            emit_g2(1)
            emit_chain(3)
            emit_reduce(2)
            emit_reduce(3)
            emit_g2(2)
            emit_g2(3)

    nc.compile()
    return nc


def host_prep(x, W_in, b_in, gamma, beta, W_out, b_out, T=N_FULL,
              og_full8=OG_FULL8):
    """Host-side input prep: casts, transposes, per-core W_in slices."""
    import ml_dtypes
    f8 = ml_dtypes.float8_e4m3fn
    x = np.asarray(x)
    gamma = np.asarray(gamma, np.float32)
    beta = np.asarray(beta, np.float32)
    W_in = np.asarray(W_in, np.float32)
    b_in = np.asarray(b_in, np.float32)
    W_out = np.asarray(W_out, np.float32)
    b_out = np.asarray(b_out, np.float32)

    xT32 = np.ascontiguousarray(
        np.asarray(x, np.float32).transpose(2, 1, 0).reshape(D, B * T))
    xT = xT32.astype(np.float16)
    xT8 = xT32.astype(f8)
    # gamma folded into W2: w2g[d, o] = gamma[d] * W_out[o, d]
    w2g = np.ascontiguousarray(gamma[:, None] * W_out.T).astype(np.float16)
    c1 = gamma @ W_out.T
    c2 = beta @ W_out.T + b_out
    c12 = np.ascontiguousarray(np.stack([c1, c2])).astype(np.float16)

    NM16 = 2 if og_full8 else 4
    in_maps = []
    for c in range(NCORES):
        base = c * 2 * P
        rows = []
        for blk in range(3):                  # inp, og, fg
            for e in range(E):                # e0, e1 (or d0, d1 for fg)
                rows.append(blk * D + base + 2 * np.arange(P) + e)
        rows = np.concatenate(rows)           # (768,)
        w1_sel = W_in[rows[:NM16 * P], :].copy()
        if not og_full8:
            w1_sel[2 * P:4 * P, :] *= 16.0    # og f16 half shares 1/16 descale
        w1T_c = np.ascontiguousarray(w1_sel.T).astype(np.float16)
        b1_c = np.ascontiguousarray(b_in[rows].reshape(M_TILES, P).T)
        w18_c = np.ascontiguousarray(
            16.0 * W_in[rows[NM16 * P:], :].T).astype(f8)
        m = {
            "xT": xT, "xT8": xT8, "w1T": w1T_c, "w18": w18_c, "b1": b1_c,
            "w2g": w2g, "c12": c12,
        }
        if not og_full8:
            m["w18og"] = np.ascontiguousarray(
                16.0 * W_in[rows[2 * P:4 * P], :D // 2].T).astype(f8)
        in_maps.append(m)
    return in_maps


def assemble_output(results, T=N_FULL):
    """Gather per-core [TOK_C, D] outputs into the full (N, B, D) array.

    Core i's local row (b*NSEG + n_loc) holds token (n = i*NSEG + n_loc, b).
    """
    NSEG = T // NCORES
    out = np.empty((T, B, D), np.float32)
    for i, res in enumerate(results):
        blk = np.asarray(res["out"], np.float32).reshape(B, NSEG, D)
        for b in range(B):
            out[i * NSEG:(i + 1) * NSEG, b, :] = blk[b]
    return out


def kernel(x, W_in, b_in, gamma, beta, W_out, b_out):
    from concourse.bass_utils import run_bass_kernel_spmd

    key = N_FULL
    if key not in _BUILD_CACHE:
        _BUILD_CACHE[key] = build_program(N_FULL)
    nc = _BUILD_CACHE[key]
    in_maps = host_prep(x, W_in, b_in, gamma, beta, W_out, b_out)
    res = run_bass_kernel_spmd(nc, in_maps, core_ids=list(range(NCORES)))
    return assemble_output(res.results)


if __name__ == "__main__":
    import reference
    inputs = {k: np.asarray(v) for k, v in reference.setup_inputs().items()}
    expected = np.asarray(reference.reference(**inputs))
    actual = kernel(**inputs)
    err = np.abs(actual - expected)
    rel = np.linalg.norm(actual - expected) / np.linalg.norm(expected)
    print("max abs err:", err.max(), "rel fro err:", rel)


# revision 22
# speedup vs baseline: 1.0427x; 1.0427x over previous
"""Bass/Trainium2 kernel for nn_BiHgru2_1d (bidirectional HGRU block), 8-core SPMD.

Math (reference):
    feat = x @ W_in.T + b_in                    # (N,B,3D)
    inp, og, fg = split(feat); inp=silu(inp); og=sigmoid(og); lam=sigmoid(fg)
    u[h,d,e] = (1-lam[h,d]) * inp[h,e];  lam_f[h,d,e] = lam[h,d]
    s = fwd_scan(lam_f, u) + rev_scan(lam_f, u)         # h_t = lam_t h_{t-1} + u_t
    o[h,e] = sum_d s[h,d,e]*og[h,d]; o = LN(o)*gamma+beta; out = o @ W_out.T + b_out

Sharding: 8-way tensor parallel over heads (128 heads/core). Each core:
  GEMM1 (x full, W_in 768-row slice; inp in fp16, og+fg in fp8 DoubleRow) ->
  activations -> per-(b,d,e) tensor_tensor_scan fwd+rev (rev via negative
  stride) -> o_acc -> per-core partial LN stats -> per-batch AllToAll
  carrying o_acc + stat partials (reshard channel->token) -> GEMM2 with
  gamma folded into W2 and the LN mean/std terms injected as a K=2
  mini-matmul row pair -> per-token scale on the ACT engine -> each core
  writes tokens (all b, n in [256i, 256(i+1))) of the output in f16.

Sign trick: we compute u' = (lam-1)*inp = -u (no rsub on HW), so s' = -s
and o' = -o.  With SUM' = sum_D o' = -D*mu and SSQ' = sum_D o'^2:
    mu = -SUM'/D, var = SSQ'/D - mu^2, sd = sqrt(var+eps), a_t = -1/sd
    out[t,:] = a_t * (G'[t,:] + mu_t*c1 - sd_t*c2)
with G' = o'@W2g (W2g = gamma-scaled W_out.T), c1 = gamma@W_out.T,
c2 = beta@W_out.T + b_out.  The (mu, -sd) pair forms a K=2 stationary row
block multiplied against (c1; c2) inside the GEMM2 accumulation, so the
epilogue is a single per-token ACT scale by a_t.

Queue schedule (in-order queues force this emission order):
  iter b: G1 blocks(b) | stats-chain(b-1) | stat-reduce(b-2) | scans(b)
  pass 2: G2(0) G2(1) | chain(3) | reduce(2) reduce(3) | G2(2) G2(3)
so no engine ever blocks another batch's producers.
"""

import sys

for _p in ("/opt/trn_rl_repo",):
    if _p not in sys.path:
        sys.path.insert(0, _p)

import numpy as np

# ---- problem constants (hardcoded per contract) ----
N_FULL, B, D = 2048, 4, 2048
E = 2
H = D // E                      # 1024 heads
NCORES = 8
P = 128                         # partitions
HC = H // NCORES                # 128 heads per core
KC = D // P                     # 16 k-chunks
M_TILES = 6                     # [inp e0, inp e1, og e0, og e1, fg d0, fg d1]

OG_FULL8 = True                 # og path fully fp8 (else half fp8 / half f16)

_BUILD_CACHE = {}


def build_program(T=N_FULL, num_devices=NCORES, og_full8=OG_FULL8,
                  use_silu=True, debug_dump=False):
    """Build the SPMD Bass program (same program on every core)."""
    import concourse.bass as bass
    import concourse.mybir as mybir
    import concourse.tile as tile
    from concourse import bacc

    f16 = mybir.dt.float16
    f32 = mybir.dt.float32
    fp8 = mybir.dt.float8e4
    MUL = mybir.AluOpType.mult
    ADD = mybir.AluOpType.add
    SUB = mybir.AluOpType.subtract
    AF = mybir.ActivationFunctionType
    DR = mybir.MatmulPerfMode.DoubleRow

    NSEG = T // NCORES           # per-core seq positions per batch (256)
    TOK_C = B * NSEG             # tokens per core after reshard (1024)
    NBLK = min(512, T)           # GEMM1 token-block size (per batch)
    NB1 = T // NBLK              # token blocks per batch (4)
    TCH = min(P, NSEG)           # GEMM2 token-chunk (output partition dim)
    NTCH = NSEG // TCH           # token chunks per batch (2)
    OCB = 256                    # GEMM2 out-col block
    NOC = D // OCB               # 8
    KD = KC // 2                 # fp8 DoubleRow k-pairs (8)
    NM16 = 2 if og_full8 else 4  # f16 m-tiles (inp; +og halves if not full8)
    M8 = 4 if og_full8 else 2    # fp8 m-tiles (og+fg or fg only)
    CH = 2 * NSEG                # stat partial chunk (512 tokens)
    NCH = T // CH                # 4
    PAY = P * E * NSEG           # oac payload elems per A2A destination
    assert T % (NCORES * TCH) == 0 and T % NBLK == 0

    nc = bacc.Bacc("TRN2", target_bir_lowering=False, debug=False,
                   num_devices=num_devices)

    # ---- per-core DRAM parameters ----
    xT_d = nc.dram_tensor("xT", [D, B * T], f16, kind="ExternalInput")
    xT8_d = nc.dram_tensor("xT8", [D, B * T], fp8, kind="ExternalInput")
    w1T_d = nc.dram_tensor("w1T", [D, NM16 * P], f16, kind="ExternalInput")
    w18_d = nc.dram_tensor("w18", [D, M8 * P], fp8, kind="ExternalInput")
    if not og_full8:
        w18og_d = nc.dram_tensor("w18og", [D // 2, 2 * P], fp8,
                                 kind="ExternalInput")
    b1_d = nc.dram_tensor("b1", [P, M_TILES], f32, kind="ExternalInput")
    w2g_d = nc.dram_tensor("w2g", [D, D], f16, kind="ExternalInput")
    c12_d = nc.dram_tensor("c12", [2, D], f16, kind="ExternalInput")
    out_d = nc.dram_tensor("out", [TOK_C, D], f16, kind="ExternalOutput")
    if debug_dump:
        dbg_act = nc.dram_tensor("dbg_act", [3, P, E, T], f16,
                                 kind="ExternalOutput")
        dbg_oac = nc.dram_tensor("dbg_oac", [B, P, E, T], f16,
                                 kind="ExternalOutput")
        dbg_ot = nc.dram_tensor("dbg_ot", [B, P, KC, NSEG], f16,
                                 kind="ExternalOutput")

    xT_r = xT_d.ap().rearrange("(kc p) t -> p kc t", p=P)
    xT8_r = xT8_d.ap().rearrange("(kd ko p) t -> p kd ko t", p=P, ko=2)
    w1T_r = w1T_d.ap().rearrange("(kc p) m -> p kc m", p=P)
    w18_r = w18_d.ap().rearrange("(kd ko p) m -> p kd ko m", p=P, ko=2)
    if not og_full8:
        w18og_r = w18og_d.ap().rearrange("(kd ko p) m -> p kd ko m", p=P, ko=2)
    w2g_r = w2g_d.ap().rearrange("(kc p) o -> p kc o", p=P)

    with tile.TileContext(nc) as tc:
        with (
            tc.tile_pool(name="cst", bufs=1) as cst_pool,
            tc.tile_pool(name="w1p", bufs=1) as w1_pool,
            tc.tile_pool(name="xs", bufs=2) as x_pool,
            tc.tile_pool(name="res", bufs=2) as res_pool,
            tc.tile_pool(name="oacp", bufs=2) as oac_pool,
            tc.tile_pool(name="up", bufs=1) as u_pool,
            tc.tile_pool(name="scan", bufs=2) as scan_pool,
            tc.tile_pool(name="otp", bufs=3) as ot_pool,
            tc.tile_pool(name="w2p", bufs=2) as w2_pool,
            tc.tile_pool(name="sqp", bufs=2) as sq_pool,
            tc.tile_pool(name="stp", bufs=2) as stp_pool,
            tc.tile_pool(name="epi", bufs=2) as epi_pool,
            tc.tile_pool(name="ps", bufs=2, space="PSUM") as psum_pool,
            tc.tile_pool(name="dram", bufs=2, space="DRAM") as dram_pool,
        ):
            # ---- constants (small, scalar queue) ----
            b1_sb = cst_pool.tile([P, M_TILES], f32, tag="b1")
            nc.scalar.dma_start(b1_sb[:], b1_d.ap())
            ones_sb = cst_pool.tile([P, 1], f16, tag="ones")
            nc.vector.memset(ones_sb[:], 1.0)
            eps_sb = cst_pool.tile([P, 1], f32, tag="eps")
            nc.vector.memset(eps_sb[:], 1e-5)
            cc12_sb = cst_pool.tile([2, D], f16, tag="cc12")
            nc.scalar.dma_start(cc12_sb[:], c12_d.ap())

            # ---- weights + batch0/block0 x, interleaved kc-wise so the
            # m-inner first block can chase the DMA stream ----
            w1_sb = w1_pool.tile([P, KC, NM16 * P], f16, tag="w1")
            w18_sb = w1_pool.tile([P, KD, 2, M8 * P], fp8, tag="w18")
            xt0 = x_pool.tile([P, KC, NBLK], f16, tag="xt", name="xt_0_0")
            xt80 = x_pool.tile([P, KD, 2, NBLK], fp8, tag="xt8", name="xt8_0_0")
            for kc in range(KC):
                nc.sync.dma_start(w1_sb[:, kc:kc + 1, :], w1T_r[:, kc:kc + 1, :])
                nc.sync.dma_start(xt0[:, kc:kc + 1, :], xT_r[:, kc:kc + 1, 0:NBLK])
                if kc % 2 == 1:
                    kd = kc // 2
                    nc.sync.dma_start(w18_sb[:, kd:kd + 1, :, :],
                                      w18_r[:, kd:kd + 1, :, :])
                    nc.sync.dma_start(xt80[:, kd:kd + 1, :, :],
                                      xT8_r[:, kd:kd + 1, :, 0:NBLK])
            if not og_full8:
                w18og_sb = w1_pool.tile([P, KD // 2, 2, 2 * P], fp8, tag="w18og")
                nc.sync.dma_start(w18og_sb[:], w18og_r)

            # per-batch live tile refs
            res_tiles = [None] * B      # (lam, inp, og, oac)
            ot_tiles = [None] * B
            stT8_tiles = [None] * B
            aT_tiles = [None] * B
            mrow_tiles = [None] * B     # [2, NTCH, TCH] f16 (mu; -sd) rows
            cc_tiles = [None] * B

            # GEMM1 m-tile epilogue: m 0,1 -> inp e0,e1 (silu);
            # 2,3 -> og e0,e1; 4,5 -> fg d0,d1 (sigmoid, 1/16 descale)
            def g1_epilogue(b, m, ps, nb):
                lam_b, inp_b, og_b, _ = res_tiles[b]
                dest, func, scale = [
                    (inp_b, AF.Silu, 1.0), (inp_b, AF.Silu, 1.0),
                    (og_b, AF.Sigmoid, 1.0 / 16.0),
                    (og_b, AF.Sigmoid, 1.0 / 16.0),
                    (lam_b, AF.Sigmoid, 1.0 / 16.0),
                    (lam_b, AF.Sigmoid, 1.0 / 16.0)][m]
                dsl = dest[:, m % 2, nb * NBLK:(nb + 1) * NBLK]
                if func == AF.Silu and not use_silu:
                    sg = sq_pool.tile([P, NBLK], f32, tag="sg")
                    nc.scalar.activation(sg[:], ps[:], AF.Sigmoid,
                                         bias=b1_sb[:, m:m + 1], scale=scale)
                    nc.scalar.activation(ps[:], ps[:], AF.Identity,
                                         bias=b1_sb[:, m:m + 1], scale=scale)
                    nc.vector.tensor_tensor(dsl, ps[:], sg[:], MUL)
                else:
                    nc.scalar.activation(dsl, ps[:], func,
                                         bias=b1_sb[:, m:m + 1], scale=scale)

            def emit_g1_block(b, nb):
                tok0 = b * T + nb * NBLK
                if b == 0 and nb == 0:
                    xt, xt8 = xt0, xt80
                else:
                    xt = x_pool.tile([P, KC, NBLK], f16, tag="xt",
                                     name=f"xt_{b}_{nb}")
                    xt8 = x_pool.tile([P, KD, 2, NBLK], fp8, tag="xt8",
                                      name=f"xt8_{b}_{nb}")
                    for q in range(4):
                        nc.sync.dma_start(
                            xt[:, 4 * q:4 * (q + 1), :],
                            xT_r[:, 4 * q:4 * (q + 1), tok0:tok0 + NBLK])
                    for q in range(2):
                        nc.sync.dma_start(
                            xt8[:, 4 * q:4 * (q + 1), :, :],
                            xT8_r[:, 4 * q:4 * (q + 1), :, tok0:tok0 + NBLK])
                if b == 0 and nb == 0:
                    # m-inner, kc-outer in two 3-tile groups: chases the
                    # interleaved DMA stream with 3 PSUM groups at once.
                    assert og_full8, "m-inner fast start needs og_full8"
                    for grp in ((0, 1, 2), (3, 4, 5)):
                        pss = {m: psum_pool.tile([P, NBLK], f32, tag="ps",
                                                 bufs=3, name=f"ps0_{m}")
                               for m in grp}
                        for kc in range(KC):
                            for m in grp:
                                if m < NM16:
                                    nc.tensor.matmul(
                                        pss[m][:],
                                        w1_sb[:, kc, m * P:(m + 1) * P],
                                        xt[:, kc, :],
                                        start=(kc == 0), stop=(kc == KC - 1))
                                elif kc % 2 == 1:
                                    kd = kc // 2
                                    m8 = m - NM16
                                    nc.tensor.matmul(
                                        pss[m][:],
                                        w18_sb[:, kd, :, m8 * P:(m8 + 1) * P],
                                        xt8[:, kd, :, :],
                                        start=(kd == 0), stop=(kd == KD - 1),
                                        perf_mode=DR)
                        for m in grp:
                            g1_epilogue(b, m, pss[m], nb)
                    return
                for m in range(M_TILES):
                    ps = psum_pool.tile([P, NBLK], f32, tag="ps", bufs=3,
                                        name=f"ps_{b}_{nb}_{m}")
                    if (og_full8 and m >= 2) or (not og_full8 and m >= 4):
                        m8 = m - NM16
                        for kd in range(KD):
                            nc.tensor.matmul(
                                ps[:], w18_sb[:, kd, :, m8 * P:(m8 + 1) * P],
                                xt8[:, kd, :, :],
                                start=(kd == 0), stop=(kd == KD - 1),
                                perf_mode=DR)
                    elif not og_full8 and m >= 2:
                        for kd in range(KD // 2):
                            nc.tensor.matmul(
                                ps[:],
                                w18og_sb[:, kd, :, (m - 2) * P:(m - 1) * P],
                                xt8[:, kd, :, :],
                                start=(kd == 0), stop=False, perf_mode=DR)
                        for kc in range(KC // 2, KC):
                            nc.tensor.matmul(
                                ps[:], w1_sb[:, kc, m * P:(m + 1) * P],
                                xt[:, kc, :], start=False, stop=(kc == KC - 1))
                    else:
                        for kc in range(KC):
                            nc.tensor.matmul(
                                ps[:], w1_sb[:, kc, m * P:(m + 1) * P],
                                xt[:, kc, :],
                                start=(kc == 0), stop=(kc == KC - 1))
                    g1_epilogue(b, m, ps, nb)

            def emit_scans(b):
                lam_b, inp_b, og_b, oac_b = res_tiles[b]
                u_t = [[u_pool.tile([P, T], f16, tag=f"u{d}{e}",
                                    name=f"u_{b}_{d}{e}")
                        for e in range(E)] for d in range(E)]
                for d in range(E):
                    for e in range(E):
                        nc.vector.scalar_tensor_tensor(
                            u_t[d][e][:], lam_b[:, d, :], 1.0, inp_b[:, e, :],
                            op0=SUB, op1=MUL)
                for rev in (False, True):
                    for d in range(E):
                        lam_bd = lam_b[:, d, :]
                        og_bd = og_b[:, d, :]
                        for e in range(E):
                            s = scan_pool.tile([P, T], f16, tag="s",
                                               name=f"s_{b}_{rev}_{d}{e}")
                            if rev:
                                nc.vector.tensor_tensor_scan(
                                    s[:, ::-1], lam_bd[:, ::-1],
                                    u_t[d][e][:, ::-1], 0.0, op0=MUL, op1=ADD)
                            else:
                                nc.vector.tensor_tensor_scan(
                                    s[:], lam_bd, u_t[d][e][:], 0.0,
                                    op0=MUL, op1=ADD)
                            o_be = oac_b[:, e, :]
                            if not rev and d == 0:
                                nc.vector.tensor_tensor(o_be, og_bd, s[:], MUL)
                            else:
                                nc.vector.tensor_tensor(s[:], og_bd, s[:], MUL)
                                nc.vector.tensor_tensor(o_be, o_be, s[:], ADD)
                if debug_dump:
                    nc.gpsimd.dma_start(dbg_oac.ap()[b], oac_b[:])
                    if b == 0:
                        nc.gpsimd.dma_start(dbg_act.ap()[0], lam_b[:])
                        nc.gpsimd.dma_start(dbg_act.ap()[1], inp_b[:])
                        nc.gpsimd.dma_start(dbg_act.ap()[2], og_b[:])

            def emit_chain(b):
                """Partial stats + A2A + ot/stat gathers for batch b.
                Emitted after G1(b+1) so the tensor/ACT queues reach the
                stat ops only once oac(b) is (nearly) ready."""
                _, inp_b, _, oac_b = res_tiles[b]
                # sqac reuses inp_b (dead once u' tiles exist)
                nc.scalar.square(inp_b[:], oac_b[:])
                cc_in = dram_pool.tile([NCORES, PAY + 2 * NSEG], f16,
                                       tag="cc_in", name=f"cc_in_{b}")
                cc_out = dram_pool.tile([NCORES, PAY + 2 * NSEG], f16,
                                        tag="cc_out", name=f"cc_out_{b}")
                cc_tiles[b] = cc_out
                cc_oac_in = cc_in[:, 0:PAY].rearrange(
                    "j (p e t) -> j p e t", p=P, e=E)
                cc_st_in = cc_in[:, PAY:].rearrange("j (s t) -> j s t", s=2)
                # stat partial matmuls (sequential accumulation groups) +
                # psum->sbuf f16 copies
                stcs = []
                for c in range(NCH):
                    ts = slice(c * CH, (c + 1) * CH)
                    for si, srct in ((0, oac_b), (1, inp_b)):
                        psc = psum_pool.tile([1, CH], f32, tag="pst", bufs=3,
                                             name=f"pst_{b}_{c}_{si}")
                        nc.tensor.matmul(psc[:], ones_sb[:], srct[:, 0, ts],
                                         start=True, stop=False)
                        nc.tensor.matmul(psc[:], ones_sb[:], srct[:, 1, ts],
                                         start=False, stop=True)
                        stc = stp_pool.tile([1, CH], f16, tag="stc", bufs=4,
                                            name=f"stc_{b}_{c}_{si}")
                        nc.vector.tensor_copy(out=stc[:], in_=psc[:])
                        stcs.append((c, si, stc))
                # gpsimd: oac staging, stat staging, collective, gathers
                for j in range(NCORES):
                    nc.gpsimd.dma_start(
                        cc_oac_in[j], oac_b[:, :, j * NSEG:(j + 1) * NSEG])
                for c, si, stc in stcs:
                    nc.gpsimd.dma_start(cc_st_in[2 * c:2 * c + 2, si, :],
                                        stc[:1, :])
                nc.gpsimd.collective_compute(
                    "AllToAll", mybir.AluOpType.bypass,
                    replica_groups=[list(range(NCORES))],
                    ins=[cc_in.opt()], outs=[cc_out.opt()])
                cc_oac_out = cc_out[:, 0:PAY].rearrange(
                    "j (h p t) -> j p h t", h=2, p=P)
                ot = ot_pool.tile([P, KC, NSEG], f16, tag="ot",
                                  name=f"ot_{b}")
                for j in range(NCORES):
                    nc.gpsimd.dma_start(ot[:, 2 * j:2 * j + 2, :],
                                        cc_oac_out[j])
                if debug_dump:
                    nc.gpsimd.dma_start(dbg_ot.ap()[b], ot[:])
                ot_tiles[b] = ot
                # stat partial gather: [tok-part, s, j, c]
                stT8 = stp_pool.tile([TCH, 2, NCORES, NTCH], f16, tag="stT8",
                                     name=f"stT8_{b}")
                for s in range(2):
                    for c in range(NTCH):
                        st8 = cc_out[:, PAY + s * NSEG + c * TCH:
                                     PAY + s * NSEG + (c + 1) * TCH].rearrange(
                            "j p -> p j", p=TCH)
                        nc.gpsimd.dma_start(stT8[:, s, :, c], st8)
                stT8_tiles[b] = stT8

            def emit_reduce(b):
                """Reduce stat partials over cores; produce aT (token-part
                scale) and the (mu, -sd) stationary rows via a DRAM bounce."""
                stT8 = stT8_tiles[b]
                nc.vector.tensor_tensor(stT8[:, :, 0:4, :], stT8[:, :, 0:4, :],
                                        stT8[:, :, 4:8, :], ADD)
                nc.vector.tensor_tensor(stT8[:, :, 0:2, :], stT8[:, :, 0:2, :],
                                        stT8[:, :, 2:4, :], ADD)
                stT = stp_pool.tile([TCH, 2, NTCH], f32, tag="stT",
                                    name=f"stT_{b}")
                nc.vector.tensor_tensor(stT[:], stT8[:, :, 0, :],
                                        stT8[:, :, 1, :], ADD)
                # mb[:, c, 0] = mu ; mb[:, c, 1] = -sd ; aT = -1/sd
                mb = stp_pool.tile([TCH, NTCH, 2], f32, tag="mb",
                                   name=f"mb_{b}")
                sd = stp_pool.tile([TCH, 2, NTCH], f32, tag="sd",
                                   name=f"sd_{b}")
                VAR, REC = 0, 1
                nc.vector.tensor_scalar(mb[:, :, 0], stT[:, 0, :], -1.0 / D,
                                        None, op0=MUL)
                nc.vector.tensor_tensor(sd[:, VAR], mb[:, :, 0], mb[:, :, 0],
                                        MUL)
                nc.vector.tensor_scalar(sd[:, REC], stT[:, 1, :], 1.0 / D,
                                        None, op0=MUL)
                nc.vector.tensor_tensor(sd[:, VAR], sd[:, REC], sd[:, VAR],
                                        SUB)
                nc.scalar.activation(sd[:, VAR], sd[:, VAR], AF.Sqrt,
                                     bias=eps_sb[:])
                nc.vector.tensor_scalar(mb[:, :, 1], sd[:, VAR], -1.0, None,
                                        op0=MUL)
                aT = stp_pool.tile([TCH, NTCH], f32, tag="aT",
                                   name=f"aT_{b}")
                nc.vector.reciprocal(sd[:, REC], sd[:, VAR])
                nc.vector.tensor_scalar(aT[:], sd[:, REC], -1.0, None,
                                        op0=MUL)
                aT_tiles[b] = aT
                # bounce (mu, -sd) to row layout [2, NTCH, TCH] f16
                mr_dram = dram_pool.tile([2, NSEG], f32, tag="mrd",
                                         name=f"mrd_{b}")
                mr_view = mr_dram.rearrange("q (c p) -> q p c", p=TCH)
                for q in range(2):
                    nc.gpsimd.dma_start(mr_view[q], mb[:, :, q])
                mrow = stp_pool.tile([2, NTCH, TCH], f16, tag="mrow",
                                     name=f"mrow_{b}")
                nc.gpsimd.dma_start(mrow[:], mr_dram.rearrange(
                    "q (c p) -> q c p", p=TCH))
                mrow_tiles[b] = mrow

            def load_w2(b, oc):
                ocs = slice(oc * OCB, (oc + 1) * OCB)
                w2sb = w2_pool.tile([P, KC, OCB], f16, tag="w2",
                                    name=f"w2_{b}_{oc}")
                for q in range(4):
                    nc.sync.dma_start(w2sb[:, 4 * q:4 * (q + 1), :],
                                      w2g_r[:, 4 * q:4 * (q + 1), ocs])
                return w2sb

            def emit_g2(b, w2_first):
                ot = ot_tiles[b]
                w2sb = w2_first
                for oc in range(NOC):
                    ocs = slice(oc * OCB, (oc + 1) * OCB)
                    if oc < NOC - 1:
                        w2_next = load_w2(b, oc + 1)
                    elif b < B - 1:
                        w2_next = load_w2(b + 1, 0)
                    else:
                        w2_next = None
                    for tch in range(NTCH):
                        ps2 = psum_pool.tile([TCH, OCB], f32, tag="ps2",
                                             bufs=2,
                                             name=f"ps2_{b}_{oc}_{tch}")
                        for kc in range(KC):
                            nc.tensor.matmul(
                                ps2[:], ot[:, kc, tch * TCH:(tch + 1) * TCH],
                                w2sb[:, kc, :], start=(kc == 0), stop=False)
                        nc.tensor.matmul(
                            ps2[:], mrow_tiles[b][:, tch, :],
                            cc12_sb[:, ocs], start=False, stop=True)
                        ob = epi_pool.tile([TCH, OCB], f16, tag="ob",
                                           name=f"ob_{b}_{oc}_{tch}")
                        nc.scalar.activation(
                            ob[:], ps2[:], AF.Copy,
                            scale=aT_tiles[b][:, tch:tch + 1])
                        nc.sync.dma_start(
                            out_d.ap()[b * NSEG + tch * TCH:
                                       b * NSEG + (tch + 1) * TCH, ocs],
                            ob[:])
                    w2sb = w2_next
                return w2_next

            # ================= emission schedule =================
            for b in range(B):
                lam_b = res_pool.tile([P, E, T], f16, tag="lam",
                                      name=f"lam_{b}")
                inp_b = res_pool.tile([P, E, T], f16, tag="inp",
                                      name=f"inp_{b}")
                og_b = res_pool.tile([P, E, T], f16, tag="og",
                                     name=f"og_{b}")
                oac_b = oac_pool.tile([P, E, T], f16, tag="oac",
                                      name=f"oac_{b}")
                res_tiles[b] = (lam_b, inp_b, og_b, oac_b)
                for nb in range(NB1):
                    emit_g1_block(b, nb)
                if b >= 1:
                    emit_chain(b - 1)
                if b >= 2:
                    emit_reduce(b - 2)
                emit_scans(b)
            w2n = load_w2(0, 0)
            w2n = emit_g2(0, w2n)
            w2n = emit_g2(1, w2n)
            emit_chain(3)
            emit_reduce(2)
            emit_reduce(3)
            w2n = emit_g2(2, w2n)
            emit_g2(3, w2n)

    nc.compile()
    return nc


def host_prep(x, W_in, b_in, gamma, beta, W_out, b_out, T=N_FULL,
              og_full8=OG_FULL8):
    """Host-side input prep: casts, transposes, per-core W_in slices."""
    import ml_dtypes
    f8 = ml_dtypes.float8_e4m3fn
    x = np.asarray(x)
    gamma = np.asarray(gamma, np.float32)
    beta = np.asarray(beta, np.float32)
    W_in = np.asarray(W_in, np.float32)
    b_in = np.asarray(b_in, np.float32)
    W_out = np.asarray(W_out, np.float32)
    b_out = np.asarray(b_out, np.float32)

    xT32 = np.ascontiguousarray(
        np.asarray(x, np.float32).transpose(2, 1, 0).reshape(D, B * T))
    xT = xT32.astype(np.float16)
    xT8 = xT32.astype(f8)
    # gamma folded into W2: w2g[d, o] = gamma[d] * W_out[o, d]
    w2g = np.ascontiguousarray(gamma[:, None] * W_out.T).astype(np.float16)
    c1 = gamma @ W_out.T
    c2 = beta @ W_out.T + b_out
    c12 = np.ascontiguousarray(np.stack([c1, c2])).astype(np.float16)

    NM16 = 2 if og_full8 else 4
    in_maps = []
    for c in range(NCORES):
        base = c * 2 * P
        rows = []
        for blk in range(3):                  # inp, og, fg
            for e in range(E):                # e0, e1 (or d0, d1 for fg)
                rows.append(blk * D + base + 2 * np.arange(P) + e)
        rows = np.concatenate(rows)           # (768,)
        w1_sel = W_in[rows[:NM16 * P], :].copy()
        if not og_full8:
            w1_sel[2 * P:4 * P, :] *= 16.0    # og f16 half shares 1/16 descale
        w1T_c = np.ascontiguousarray(w1_sel.T).astype(np.float16)
        b1_c = np.ascontiguousarray(b_in[rows].reshape(M_TILES, P).T)
        w18_c = np.ascontiguousarray(
            16.0 * W_in[rows[NM16 * P:], :].T).astype(f8)
        m = {
            "xT": xT, "xT8": xT8, "w1T": w1T_c, "w18": w18_c, "b1": b1_c,
            "w2g": w2g, "c12": c12,
        }
        if not og_full8:
            m["w18og"] = np.ascontiguousarray(
                16.0 * W_in[rows[2 * P:4 * P], :D // 2].T).astype(f8)
        in_maps.append(m)
    return in_maps


def assemble_output(results, T=N_FULL):
    """Gather per-core [TOK_C, D] outputs into the full (N, B, D) array.

    Core i's local row (b*NSEG + n_loc) holds token (n = i*NSEG + n_loc, b).
    """
    NSEG = T // NCORES
    out = np.empty((T, B, D), np.float32)
    for i, res in enumerate(results):
        blk = np.asarray(res["out"], np.float32).reshape(B, NSEG, D)
        for b in range(B):
            out[i * NSEG:(i + 1) * NSEG, b, :] = blk[b]
    return out


def kernel(x, W_in, b_in, gamma, beta, W_out, b_out):
    from concourse.bass_utils import run_bass_kernel_spmd

    key = N_FULL
    if key not in _BUILD_CACHE:
        _BUILD_CACHE[key] = build_program(N_FULL)
    nc = _BUILD_CACHE[key]
    in_maps = host_prep(x, W_in, b_in, gamma, beta, W_out, b_out)
    res = run_bass_kernel_spmd(nc, in_maps, core_ids=list(range(NCORES)))
    return assemble_output(res.results)


if __name__ == "__main__":
    import reference
    inputs = {k: np.asarray(v) for k, v in reference.setup_inputs().items()}
    expected = np.asarray(reference.reference(**inputs))
    actual = kernel(**inputs)
    err = np.abs(actual - expected)
    rel = np.linalg.norm(actual - expected) / np.linalg.norm(expected)
    print("max abs err:", err.max(), "rel fro err:", rel)


# revision 25
# speedup vs baseline: 1.1552x; 1.1079x over previous
"""Bass/Trainium2 kernel for nn_BiHgru2_1d (bidirectional HGRU block), 8-core SPMD.

Math (reference):
    feat = x @ W_in.T + b_in                    # (N,B,3D)
    inp, og, fg = split(feat); inp=silu(inp); og=sigmoid(og); lam=sigmoid(fg)
    u[h,d,e] = (1-lam[h,d]) * inp[h,e];  lam_f[h,d,e] = lam[h,d]
    s = fwd_scan(lam_f, u) + rev_scan(lam_f, u)         # h_t = lam_t h_{t-1} + u_t
    o[h,e] = sum_d s[h,d,e]*og[h,d]; o = LN(o)*gamma+beta; out = o @ W_out.T + b_out

Sharding: 8-way tensor parallel over heads (128 heads/core). Each core:
  GEMM1 (x full, W_in 768-row slice; inp in fp16, og+fg in fp8 DoubleRow) ->
  activations -> per-(b,d,e) tensor_tensor_scan fwd+rev (rev via negative
  stride) -> o_acc -> per-core partial LN stats -> per-batch AllToAll
  carrying o_acc + stat partials (reshard channel->token) -> GEMM2 with
  gamma folded into W2 and the LN mean/std terms injected as a K=2
  mini-matmul row pair -> per-token scale on the ACT engine -> each core
  writes tokens (all b, n in [256i, 256(i+1))) of the output in f16.

Sign trick: we compute u' = (lam-1)*inp = -u (no rsub on HW), so s' = -s
and o' = -o.  With SUM' = sum_D o' = -D*mu and SSQ' = sum_D o'^2:
    mu = -SUM'/D, var = SSQ'/D - mu^2, sd = sqrt(var+eps), a_t = -1/sd
    out[t,:] = a_t * (G'[t,:] + mu_t*c1 - sd_t*c2)
with G' = o'@W2g (W2g = gamma-scaled W_out.T), c1 = gamma@W_out.T,
c2 = beta@W_out.T + b_out.  The (mu, -sd) pair forms a K=2 stationary row
block multiplied against (c1; c2) inside the GEMM2 accumulation, so the
epilogue is a single per-token ACT scale by a_t.

Queue schedule (in-order queues force this emission order):
  iter b: G1 blocks(b) | stats-chain(b-1) | stat-reduce(b-2) | scans(b)
  pass 2: G2(0) G2(1) | chain(3) | reduce(2) reduce(3) | G2(2) G2(3)
so no engine ever blocks another batch's producers.
"""

import sys

for _p in ("/opt/trn_rl_repo",):
    if _p not in sys.path:
        sys.path.insert(0, _p)

import numpy as np

# ---- problem constants (hardcoded per contract) ----
N_FULL, B, D = 2048, 4, 2048
E = 2
H = D // E                      # 1024 heads
NCORES = 8
P = 128                         # partitions
HC = H // NCORES                # 128 heads per core
KC = D // P                     # 16 k-chunks
M_TILES = 6                     # [inp e0, inp e1, og e0, og e1, fg d0, fg d1]

OG_FULL8 = True                 # og path fully fp8 (else half fp8 / half f16)

_BUILD_CACHE = {}


def build_program(T=N_FULL, num_devices=NCORES, og_full8=OG_FULL8,
                  use_silu=True, debug_dump=False):
    """Build the SPMD Bass program (same program on every core)."""
    import concourse.bass as bass
    import concourse.mybir as mybir
    import concourse.tile as tile
    from concourse import bacc

    f16 = mybir.dt.float16
    f32 = mybir.dt.float32
    fp8 = mybir.dt.float8e4
    MUL = mybir.AluOpType.mult
    ADD = mybir.AluOpType.add
    SUB = mybir.AluOpType.subtract
    AF = mybir.ActivationFunctionType
    DR = mybir.MatmulPerfMode.DoubleRow

    NSEG = T // NCORES           # per-core seq positions per batch (256)
    TOK_C = B * NSEG             # tokens per core after reshard (1024)
    NBLK = min(512, T)           # GEMM1 token-block size (per batch)
    NB1 = T // NBLK              # token blocks per batch (4)
    TCH = min(P, NSEG)           # GEMM2 token-chunk (output partition dim)
    NTCH = NSEG // TCH           # token chunks per batch (2)
    OCB = 256                    # GEMM2 out-col block
    NOC = D // OCB               # 8
    KD = KC // 2                 # fp8 DoubleRow k-pairs (8)
    NM16 = 2 if og_full8 else 4  # f16 m-tiles (inp; +og halves if not full8)
    M8 = 4 if og_full8 else 2    # fp8 m-tiles (og+fg or fg only)
    CH = 2 * NSEG                # stat partial chunk (512 tokens)
    NCH = T // CH                # 4
    PAY = P * E * NSEG           # oac payload elems per A2A destination
    assert T % (NCORES * TCH) == 0 and T % NBLK == 0

    nc = bacc.Bacc("TRN2", target_bir_lowering=False, debug=False,
                   num_devices=num_devices)

    # ---- per-core DRAM parameters ----
    xT_d = nc.dram_tensor("xT", [D, B * T], f16, kind="ExternalInput")
    xT8_d = nc.dram_tensor("xT8", [D, B * T], fp8, kind="ExternalInput")
    w1T_d = nc.dram_tensor("w1T", [D, NM16 * P], f16, kind="ExternalInput")
    w18_d = nc.dram_tensor("w18", [D, M8 * P], fp8, kind="ExternalInput")
    if not og_full8:
        w18og_d = nc.dram_tensor("w18og", [D // 2, 2 * P], fp8,
                                 kind="ExternalInput")
    b1_d = nc.dram_tensor("b1", [P, M_TILES], f32, kind="ExternalInput")
    w2g_d = nc.dram_tensor("w2g", [D, D], f16, kind="ExternalInput")
    c12_d = nc.dram_tensor("c12", [2, D], f16, kind="ExternalInput")
    ident_d = nc.dram_tensor("ident", [P, P], f16, kind="ExternalInput")
    out_d = nc.dram_tensor("out", [TOK_C, D], f16, kind="ExternalOutput")
    if debug_dump:
        dbg_act = nc.dram_tensor("dbg_act", [3, P, E, T], f16,
                                 kind="ExternalOutput")
        dbg_oac = nc.dram_tensor("dbg_oac", [B, P, E, T], f16,
                                 kind="ExternalOutput")
        dbg_ot = nc.dram_tensor("dbg_ot", [B, P, KC, NSEG], f16,
                                 kind="ExternalOutput")

    xT_r = xT_d.ap().rearrange("(kc p) t -> p kc t", p=P)
    xT8_r = xT8_d.ap().rearrange("(kd ko p) t -> p kd ko t", p=P, ko=2)
    w1T_r = w1T_d.ap().rearrange("(kc p) m -> p kc m", p=P)
    w18_r = w18_d.ap().rearrange("(kd ko p) m -> p kd ko m", p=P, ko=2)
    if not og_full8:
        w18og_r = w18og_d.ap().rearrange("(kd ko p) m -> p kd ko m", p=P, ko=2)
    w2g_r = w2g_d.ap().rearrange("(kc p) o -> p kc o", p=P)

    with tile.TileContext(nc) as tc:
        with (
            tc.tile_pool(name="cst", bufs=1) as cst_pool,
            tc.tile_pool(name="w1p", bufs=1) as w1_pool,
            tc.tile_pool(name="xs", bufs=2) as x_pool,
            tc.tile_pool(name="res", bufs=2) as res_pool,
            tc.tile_pool(name="oacp", bufs=2) as oac_pool,
            tc.tile_pool(name="up", bufs=1) as u_pool,
            tc.tile_pool(name="scan", bufs=2) as scan_pool,
            tc.tile_pool(name="otp", bufs=3) as ot_pool,
            tc.tile_pool(name="w2p", bufs=2) as w2_pool,
            tc.tile_pool(name="sqp", bufs=2) as sq_pool,
            tc.tile_pool(name="stp", bufs=2) as stp_pool,
            tc.tile_pool(name="epi", bufs=2) as epi_pool,
            tc.tile_pool(name="ps", bufs=2, space="PSUM") as psum_pool,
            tc.tile_pool(name="dram", bufs=2, space="DRAM") as dram_pool,
        ):
            # ---- constants (small, scalar queue) ----
            b1_sb = cst_pool.tile([P, M_TILES], f32, tag="b1")
            nc.scalar.dma_start(b1_sb[:], b1_d.ap())
            ones_sb = cst_pool.tile([P, 1], f16, tag="ones")
            nc.vector.memset(ones_sb[:], 1.0)
            eps_sb = cst_pool.tile([P, 1], f32, tag="eps")
            nc.vector.memset(eps_sb[:], 1e-5)
            cc12_sb = cst_pool.tile([2, D], f16, tag="cc12")
            nc.scalar.dma_start(cc12_sb[:], c12_d.ap())
            ident_sb = cst_pool.tile([P, P], f16, tag="ident")
            nc.scalar.dma_start(ident_sb[:], ident_d.ap())

            # ---- weights + batch0/block0 x, interleaved kc-wise so the
            # m-inner first block can chase the DMA stream ----
            w1_sb = w1_pool.tile([P, KC, NM16 * P], f16, tag="w1")
            w18_sb = w1_pool.tile([P, KD, 2, M8 * P], fp8, tag="w18")
            xt0 = x_pool.tile([P, KC, NBLK], f16, tag="xt", name="xt_0_0")
            xt80 = x_pool.tile([P, KD, 2, NBLK], fp8, tag="xt8", name="xt8_0_0")
            for kc in range(KC):
                nc.sync.dma_start(w1_sb[:, kc:kc + 1, :], w1T_r[:, kc:kc + 1, :])
                nc.sync.dma_start(xt0[:, kc:kc + 1, :], xT_r[:, kc:kc + 1, 0:NBLK])
                if kc % 2 == 1:
                    kd = kc // 2
                    nc.sync.dma_start(w18_sb[:, kd:kd + 1, :, :],
                                      w18_r[:, kd:kd + 1, :, :])
                    nc.sync.dma_start(xt80[:, kd:kd + 1, :, :],
                                      xT8_r[:, kd:kd + 1, :, 0:NBLK])
            if not og_full8:
                w18og_sb = w1_pool.tile([P, KD // 2, 2, 2 * P], fp8, tag="w18og")
                nc.sync.dma_start(w18og_sb[:], w18og_r)

            # per-batch live tile refs
            res_tiles = [None] * B      # (lam, inp, og, oac)
            ot_tiles = [None] * B
            sum_tiles = [None] * B
            sumT_tiles = [None] * B
            ssqa_tiles = [None] * B
            aT_tiles = [None] * B
            mrow_tiles = [None] * B     # [2, NTCH, TCH] f16 (mu; -sd) rows
            cc_tiles = [None] * B

            # GEMM1 m-tile epilogue: m 0,1 -> inp e0,e1 (silu);
            # 2,3 -> og e0,e1; 4,5 -> fg d0,d1 (sigmoid, 1/16 descale)
            def g1_epilogue(b, m, ps, nb):
                lam_b, inp_b, og_b, _ = res_tiles[b]
                dest, func, scale = [
                    (inp_b, AF.Silu, 1.0), (inp_b, AF.Silu, 1.0),
                    (og_b, AF.Sigmoid, 1.0 / 16.0),
                    (og_b, AF.Sigmoid, 1.0 / 16.0),
                    (lam_b, AF.Sigmoid, 1.0 / 16.0),
                    (lam_b, AF.Sigmoid, 1.0 / 16.0)][m]
                dsl = dest[:, m % 2, nb * NBLK:(nb + 1) * NBLK]
                if func == AF.Silu and not use_silu:
                    sg = sq_pool.tile([P, NBLK], f32, tag="sg")
                    nc.scalar.activation(sg[:], ps[:], AF.Sigmoid,
                                         bias=b1_sb[:, m:m + 1], scale=scale)
                    nc.scalar.activation(ps[:], ps[:], AF.Identity,
                                         bias=b1_sb[:, m:m + 1], scale=scale)
                    nc.vector.tensor_tensor(dsl, ps[:], sg[:], MUL)
                else:
                    nc.scalar.activation(dsl, ps[:], func,
                                         bias=b1_sb[:, m:m + 1], scale=scale)

            def emit_g1_block(b, nb):
                tok0 = b * T + nb * NBLK
                if b == 0 and nb == 0:
                    xt, xt8 = xt0, xt80
                else:
                    xt = x_pool.tile([P, KC, NBLK], f16, tag="xt",
                                     name=f"xt_{b}_{nb}")
                    xt8 = x_pool.tile([P, KD, 2, NBLK], fp8, tag="xt8",
                                      name=f"xt8_{b}_{nb}")
                    for q in range(4):
                        nc.sync.dma_start(
                            xt[:, 4 * q:4 * (q + 1), :],
                            xT_r[:, 4 * q:4 * (q + 1), tok0:tok0 + NBLK])
                    for q in range(2):
                        nc.sync.dma_start(
                            xt8[:, 4 * q:4 * (q + 1), :, :],
                            xT8_r[:, 4 * q:4 * (q + 1), :, tok0:tok0 + NBLK])
                if b == 0 and nb == 0:
                    # m-inner, kc-outer in two 3-tile groups: chases the
                    # interleaved DMA stream with 3 PSUM groups at once.
                    assert og_full8, "m-inner fast start needs og_full8"
                    for grp in ((0, 1, 2), (3, 4, 5)):
                        pss = {m: psum_pool.tile([P, NBLK], f32, tag="ps",
                                                 bufs=3, name=f"ps0_{m}")
                               for m in grp}
                        for kc in range(KC):
                            for m in grp:
                                if m < NM16:
                                    nc.tensor.matmul(
                                        pss[m][:],
                                        w1_sb[:, kc, m * P:(m + 1) * P],
                                        xt[:, kc, :],
                                        start=(kc == 0), stop=(kc == KC - 1))
                                elif kc % 2 == 1:
                                    kd = kc // 2
                                    m8 = m - NM16
                                    nc.tensor.matmul(
                                        pss[m][:],
                                        w18_sb[:, kd, :, m8 * P:(m8 + 1) * P],
                                        xt8[:, kd, :, :],
                                        start=(kd == 0), stop=(kd == KD - 1),
                                        perf_mode=DR)
                        for m in grp:
                            g1_epilogue(b, m, pss[m], nb)
                    return
                for m in range(M_TILES):
                    ps = psum_pool.tile([P, NBLK], f32, tag="ps", bufs=3,
                                        name=f"ps_{b}_{nb}_{m}")
                    if (og_full8 and m >= 2) or (not og_full8 and m >= 4):
                        m8 = m - NM16
                        for kd in range(KD):
                            nc.tensor.matmul(
                                ps[:], w18_sb[:, kd, :, m8 * P:(m8 + 1) * P],
                                xt8[:, kd, :, :],
                                start=(kd == 0), stop=(kd == KD - 1),
                                perf_mode=DR)
                    elif not og_full8 and m >= 2:
                        for kd in range(KD // 2):
                            nc.tensor.matmul(
                                ps[:],
                                w18og_sb[:, kd, :, (m - 2) * P:(m - 1) * P],
                                xt8[:, kd, :, :],
                                start=(kd == 0), stop=False, perf_mode=DR)
                        for kc in range(KC // 2, KC):
                            nc.tensor.matmul(
                                ps[:], w1_sb[:, kc, m * P:(m + 1) * P],
                                xt[:, kc, :], start=False, stop=(kc == KC - 1))
                    else:
                        for kc in range(KC):
                            nc.tensor.matmul(
                                ps[:], w1_sb[:, kc, m * P:(m + 1) * P],
                                xt[:, kc, :],
                                start=(kc == 0), stop=(kc == KC - 1))
                    g1_epilogue(b, m, ps, nb)

            def emit_scans(b):
                lam_b, inp_b, og_b, oac_b = res_tiles[b]
                u_t = [[u_pool.tile([P, T], f16, tag=f"u{d}{e}",
                                    name=f"u_{b}_{d}{e}")
                        for e in range(E)] for d in range(E)]
                for d in range(E):
                    for e in range(E):
                        nc.vector.scalar_tensor_tensor(
                            u_t[d][e][:], lam_b[:, d, :], 1.0, inp_b[:, e, :],
                            op0=SUB, op1=MUL)
                for rev in (False, True):
                    for d in range(E):
                        lam_bd = lam_b[:, d, :]
                        og_bd = og_b[:, d, :]
                        for e in range(E):
                            s = scan_pool.tile([P, T], f16, tag="s",
                                               name=f"s_{b}_{rev}_{d}{e}")
                            if rev:
                                nc.vector.tensor_tensor_scan(
                                    s[:, ::-1], lam_bd[:, ::-1],
                                    u_t[d][e][:, ::-1], 0.0, op0=MUL, op1=ADD)
                            else:
                                nc.vector.tensor_tensor_scan(
                                    s[:], lam_bd, u_t[d][e][:], 0.0,
                                    op0=MUL, op1=ADD)
                            o_be = oac_b[:, e, :]
                            if not rev and d == 0:
                                nc.vector.tensor_tensor(o_be, og_bd, s[:], MUL)
                            else:
                                nc.vector.tensor_tensor(s[:], og_bd, s[:], MUL)
                                nc.vector.tensor_tensor(o_be, o_be, s[:], ADD)
                if debug_dump:
                    nc.gpsimd.dma_start(dbg_oac.ap()[b], oac_b[:])
                    if b == 0:
                        nc.gpsimd.dma_start(dbg_act.ap()[0], lam_b[:])
                        nc.gpsimd.dma_start(dbg_act.ap()[1], inp_b[:])
                        nc.gpsimd.dma_start(dbg_act.ap()[2], og_b[:])

            def emit_chain(b):
                """A2A for batch b: oac payload only, triggered as soon as
                the staging DMAs (gated on oac) complete."""
                _, _, _, oac_b = res_tiles[b]
                cc_in = dram_pool.tile([NCORES, PAY], f16,
                                       tag="cc_in", name=f"cc_in_{b}")
                cc_out = dram_pool.tile([NCORES, PAY], f16,
                                        tag="cc_out", name=f"cc_out_{b}")
                cc_tiles[b] = cc_out
                cc_oac_in = cc_in.rearrange("j (p e t) -> j p e t", p=P, e=E)
                for j in range(NCORES):
                    nc.gpsimd.dma_start(
                        cc_oac_in[j], oac_b[:, :, j * NSEG:(j + 1) * NSEG])
                nc.gpsimd.collective_compute(
                    "AllToAll", mybir.AluOpType.bypass,
                    replica_groups=[list(range(NCORES))],
                    ins=[cc_in.opt()], outs=[cc_out.opt()])
                cc_oac_out = cc_out.rearrange(
                    "j (h p t) -> j p h t", h=2, p=P)
                ot = ot_pool.tile([P, KC, NSEG], f16, tag="ot",
                                  name=f"ot_{b}")
                for j in range(NCORES):
                    nc.gpsimd.dma_start(ot[:, 2 * j:2 * j + 2, :],
                                        cc_oac_out[j])
                if debug_dump:
                    nc.gpsimd.dma_start(dbg_ot.ap()[b], ot[:])
                ot_tiles[b] = ot

            def emit_stats_t(b):
                """Tensor/ACT side of post-A2A stats for batch b:
                SUM' via ones-matmuls (row layout); SSQ' via PE-transpose
                chunks + ACT Square accumulate (token layout)."""
                ot = ot_tiles[b]
                sum_row = stp_pool.tile([1, NSEG], f32, tag="sumrow",
                                        name=f"sumrow_{b}")
                pss = psum_pool.tile([1, NSEG], f32, tag="pst", bufs=1,
                                     name=f"pss_{b}")
                for kc in range(KC):
                    nc.tensor.matmul(pss[:], ones_sb[:], ot[:, kc, :],
                                     start=(kc == 0), stop=(kc == KC - 1))
                nc.scalar.copy(sum_row[:], pss[:])
                sum_tiles[b] = sum_row
                # row -> token bounce for mu
                sr_dram = dram_pool.tile([1, NSEG], f32, tag="srd",
                                         name=f"srd_{b}")
                nc.gpsimd.dma_start(sr_dram[:], sum_row[:])
                sumT = stp_pool.tile([TCH, NTCH], f32, tag="sumT",
                                     name=f"sumT_{b}")
                nc.gpsimd.dma_start(sumT[:],
                                    sr_dram.rearrange("one (c p) -> (one p) c",
                                                      p=TCH))
                sumT_tiles[b] = sumT
                # SSQ': per (tch, group of 4 kc): 4 transposes + 1 ACT
                # Square with accum_out
                ssqa = stp_pool.tile([TCH, NTCH, 4], f32, tag="ssqa",
                                     name=f"ssqa_{b}")
                for tch in range(NTCH):
                    for g in range(4):
                        pt = psum_pool.tile([TCH, 4, P], f16, tag="pt",
                                            bufs=2, name=f"pt_{b}_{tch}_{g}")
                        for k in range(4):
                            kc = g * 4 + k
                            nc.tensor.transpose(
                                pt[:, k, :],
                                ot[:, kc, tch * TCH:(tch + 1) * TCH],
                                ident_sb[:])
                        scr = sq_pool.tile([TCH, 4 * P], f16, tag="scr",
                                           name=f"scr_{b}_{tch}_{g}")
                        nc.scalar.activation(scr[:], pt[:], AF.Square,
                                             accum_out=ssqa[:, tch,
                                                           g:g + 1])
                ssqa_tiles[b] = ssqa

            def emit_stats_v(b):
                """Vector-side stat math for batch b (tiny ops, emitted
                after the next batch's scans so DVE is not blocked)."""
                ssqr = stp_pool.tile([TCH, NTCH], f32, tag="ssqr",
                                     name=f"ssqr_{b}")
                nc.vector.tensor_reduce(ssqr[:], ssqa_tiles[b][:],
                                        mybir.AxisListType.X,
                                        mybir.AluOpType.add)
                # mu = -SUM'/D (token layout); mrow[0] = mu (row layout)
                mrow = stp_pool.tile([2, NTCH, TCH], f16, tag="mrow",
                                     name=f"mrow_{b}")
                nc.vector.tensor_scalar(
                    mrow[0:1, :, :],
                    sum_tiles[b].rearrange("one (c t) -> one c t", c=NTCH),
                    -1.0 / D, None, op0=MUL)
                mrow_tiles[b] = mrow
                sd = stp_pool.tile([TCH, 4, NTCH], f32, tag="sd",
                                   name=f"sd_{b}")
                MU, VAR, REC, MSD = range(4)
                nc.vector.tensor_scalar(sd[:, MU], sumT_tiles[b][:],
                                        -1.0 / D, None, op0=MUL)
                nc.vector.tensor_tensor(sd[:, VAR], sd[:, MU], sd[:, MU],
                                        MUL)
                nc.vector.tensor_scalar(sd[:, REC], ssqr[:], 1.0 / D,
                                        None, op0=MUL)
                nc.vector.tensor_tensor(sd[:, VAR], sd[:, REC], sd[:, VAR],
                                        SUB)
                nc.scalar.activation(sd[:, VAR], sd[:, VAR], AF.Sqrt,
                                     bias=eps_sb[:])
                nc.vector.tensor_scalar(sd[:, MSD], sd[:, VAR], -1.0, None,
                                        op0=MUL)
                aT = stp_pool.tile([TCH, NTCH], f32, tag="aT",
                                   name=f"aT_{b}")
                nc.vector.reciprocal(sd[:, REC], sd[:, VAR])
                nc.vector.tensor_scalar(aT[:], sd[:, REC], -1.0, None,
                                        op0=MUL)
                aT_tiles[b] = aT
                # bounce -sd (token) -> mrow[1] (row layout)
                ms_dram = dram_pool.tile([1, NSEG], f32, tag="msd",
                                         name=f"msd_{b}")
                nc.gpsimd.dma_start(
                    ms_dram.rearrange("one (c p) -> (one p) c", p=TCH),
                    sd[:, MSD])
                nc.gpsimd.dma_start(
                    mrow[1:2, :, :],
                    ms_dram.rearrange("one (c p) -> one c p", p=TCH))

            def load_w2(b, oc):
                ocs = slice(oc * OCB, (oc + 1) * OCB)
                w2sb = w2_pool.tile([P, KC, OCB], f16, tag="w2",
                                    name=f"w2_{b}_{oc}")
                for q in range(4):
                    nc.sync.dma_start(w2sb[:, 4 * q:4 * (q + 1), :],
                                      w2g_r[:, 4 * q:4 * (q + 1), ocs])
                return w2sb

            def emit_g2(b, w2_first):
                ot = ot_tiles[b]
                w2sb = w2_first
                for oc in range(NOC):
                    ocs = slice(oc * OCB, (oc + 1) * OCB)
                    if oc < NOC - 1:
                        w2_next = load_w2(b, oc + 1)
                    elif b < B - 1:
                        w2_next = load_w2(b + 1, 0)
                    else:
                        w2_next = None
                    for tch in range(NTCH):
                        ps2 = psum_pool.tile([TCH, OCB], f32, tag="ps2",
                                             bufs=2,
                                             name=f"ps2_{b}_{oc}_{tch}")
                        for kc in range(KC):
                            nc.tensor.matmul(
                                ps2[:], ot[:, kc, tch * TCH:(tch + 1) * TCH],
                                w2sb[:, kc, :], start=(kc == 0), stop=False)
                        nc.tensor.matmul(
                            ps2[:], mrow_tiles[b][:, tch, :],
                            cc12_sb[:, ocs], start=False, stop=True)
                        ob = epi_pool.tile([TCH, OCB], f16, tag="ob",
                                           name=f"ob_{b}_{oc}_{tch}")
                        nc.scalar.activation(
                            ob[:], ps2[:], AF.Copy,
                            scale=aT_tiles[b][:, tch:tch + 1])
                        nc.sync.dma_start(
                            out_d.ap()[b * NSEG + tch * TCH:
                                       b * NSEG + (tch + 1) * TCH, ocs],
                            ob[:])
                    w2sb = w2_next
                return w2_next

            # ================= emission schedule =================
            for b in range(B):
                lam_b = res_pool.tile([P, E, T], f16, tag="lam",
                                      name=f"lam_{b}")
                inp_b = res_pool.tile([P, E, T], f16, tag="inp",
                                      name=f"inp_{b}")
                og_b = res_pool.tile([P, E, T], f16, tag="og",
                                     name=f"og_{b}")
                oac_b = oac_pool.tile([P, E, T], f16, tag="oac",
                                      name=f"oac_{b}")
                res_tiles[b] = (lam_b, inp_b, og_b, oac_b)
                for nb in range(NB1):
                    emit_g1_block(b, nb)
                if b >= 1:
                    emit_stats_t(b - 1)
                emit_scans(b)
                emit_chain(b)
                if b >= 1:
                    emit_stats_v(b - 1)
            w2n = load_w2(0, 0)
            w2n = emit_g2(0, w2n)
            w2n = emit_g2(1, w2n)
            emit_stats_t(3)
            emit_stats_v(3)
            w2n = emit_g2(2, w2n)
            emit_g2(3, w2n)

    nc.compile()
    return nc


def host_prep(x, W_in, b_in, gamma, beta, W_out, b_out, T=N_FULL,
              og_full8=OG_FULL8):
    """Host-side input prep: casts, transposes, per-core W_in slices."""
    import ml_dtypes
    f8 = ml_dtypes.float8_e4m3fn
    x = np.asarray(x)
    gamma = np.asarray(gamma, np.float32)
    beta = np.asarray(beta, np.float32)
    W_in = np.asarray(W_in, np.float32)
    b_in = np.asarray(b_in, np.float32)
    W_out = np.asarray(W_out, np.float32)
    b_out = np.asarray(b_out, np.float32)

    xT32 = np.ascontiguousarray(
        np.asarray(x, np.float32).transpose(2, 1, 0).reshape(D, B * T))
    xT = xT32.astype(np.float16)
    xT8 = xT32.astype(f8)
    # gamma folded into W2: w2g[d, o] = gamma[d] * W_out[o, d]
    w2g = np.ascontiguousarray(gamma[:, None] * W_out.T).astype(np.float16)
    c1 = gamma @ W_out.T
    c2 = beta @ W_out.T + b_out
    c12 = np.ascontiguousarray(np.stack([c1, c2])).astype(np.float16)

    NM16 = 2 if og_full8 else 4
    in_maps = []
    for c in range(NCORES):
        base = c * 2 * P
        rows = []
        for blk in range(3):                  # inp, og, fg
            for e in range(E):                # e0, e1 (or d0, d1 for fg)
                rows.append(blk * D + base + 2 * np.arange(P) + e)
        rows = np.concatenate(rows)           # (768,)
        w1_sel = W_in[rows[:NM16 * P], :].copy()
        if not og_full8:
            w1_sel[2 * P:4 * P, :] *= 16.0    # og f16 half shares 1/16 descale
        w1T_c = np.ascontiguousarray(w1_sel.T).astype(np.float16)
        b1_c = np.ascontiguousarray(b_in[rows].reshape(M_TILES, P).T)
        w18_c = np.ascontiguousarray(
            16.0 * W_in[rows[NM16 * P:], :].T).astype(f8)
        m = {
            "xT": xT, "xT8": xT8, "w1T": w1T_c, "w18": w18_c, "b1": b1_c,
            "w2g": w2g, "c12": c12, "ident": np.eye(P, dtype=np.float16),
        }
        if not og_full8:
            m["w18og"] = np.ascontiguousarray(
                16.0 * W_in[rows[2 * P:4 * P], :D // 2].T).astype(f8)
        in_maps.append(m)
    return in_maps


def assemble_output(results, T=N_FULL):
    """Gather per-core [TOK_C, D] outputs into the full (N, B, D) array.

    Core i's local row (b*NSEG + n_loc) holds token (n = i*NSEG + n_loc, b).
    """
    NSEG = T // NCORES
    out = np.empty((T, B, D), np.float32)
    for i, res in enumerate(results):
        blk = np.asarray(res["out"], np.float32).reshape(B, NSEG, D)
        for b in range(B):
            out[i * NSEG:(i + 1) * NSEG, b, :] = blk[b]
    return out


def kernel(x, W_in, b_in, gamma, beta, W_out, b_out):
    from concourse.bass_utils import run_bass_kernel_spmd

    key = N_FULL
    if key not in _BUILD_CACHE:
        _BUILD_CACHE[key] = build_program(N_FULL)
    nc = _BUILD_CACHE[key]
    in_maps = host_prep(x, W_in, b_in, gamma, beta, W_out, b_out)
    res = run_bass_kernel_spmd(nc, in_maps, core_ids=list(range(NCORES)))
    return assemble_output(res.results)


if __name__ == "__main__":
    import reference
    inputs = {k: np.asarray(v) for k, v in reference.setup_inputs().items()}
    expected = np.asarray(reference.reference(**inputs))
    actual = kernel(**inputs)
    err = np.abs(actual - expected)
    rel = np.linalg.norm(actual - expected) / np.linalg.norm(expected)
    print("max abs err:", err.max(), "rel fro err:", rel)
